# revision 16
# baseline (speedup 1.0000x reference)
"""AttentionBlock (GroupNorm + single-head-dim-64 MHA + proj + residual) on 8 trn2 cores.

Data-parallel over batch: 16 batches -> 2 per core. Params replicated.
All matmuls in fp32r (full PE rate at N=512, ~tf32 precision), softmax in fp32.
"""

import ml_dtypes
import numpy as np

import concourse.bass as bass
import concourse.bacc as bacc
import concourse.mybir as mybir
import concourse.tile as tile

F32 = mybir.dt.float32
F32R = mybir.dt.float32r
BF16 = mybir.dt.bfloat16
ALU = mybir.AluOpType
AFT = mybir.ActivationFunctionType

B_TOT, C, T = 16, 512, 1024  # T = H*W = 32*32
N_CORES = 8
B_LOC = B_TOT // N_CORES  # 2 batches per core
NG = 32  # groups
NH = 8  # heads
CH = 64  # head channels
EPS = 1e-5
SCALE2 = 0.125  # (1/sqrt(sqrt(64)))**2 applied to q@k scores


def _emit(nc, tc, pools, x_ap, out_ap, dram):
    (consts, xb, xnp, vtp, qkp, ep, apool, opool, rpool, rbp, gnp,
     scp, avp, zpp, msp) = pools

    # ---- load constants -------------------------------------------------
    wq = []
    for j in range(4):
        wqt = consts.tile([128, 3 * C], F32R, tag=f"wq{j}", name=f"wq{j}")
        nc.sync.dma_start(out=wqt, in_=dram["wqkvT"][128 * j:128 * (j + 1), :])
        wq.append(wqt)
    wp = []
    for j in range(4):
        wpt = consts.tile([128, C], F32R, tag=f"wp{j}", name=f"wp{j}")
        nc.sync.dma_start(out=wpt, in_=dram["wprojT"][128 * j:128 * (j + 1), :])
        wp.append(wpt)
    qb_s = consts.tile([128, 8], F32, tag="qb", name="qb_s")
    nc.sync.dma_start(out=qb_s, in_=dram["qb"][:, :])
    vb_s = consts.tile([128, 512], F32, tag="vb", name="vb_s")
    nc.sync.dma_start(out=vb_s, in_=dram["vb"][:, :])
    pb_s = consts.tile([1, 512], F32R, tag="pb", name="pb_s")
    nc.sync.dma_start(out=pb_s, in_=dram["pb"][:, :])
    nwb_s = consts.tile([128, 8], F32, tag="nwb", name="nwb_s")
    nc.sync.dma_start(out=nwb_s, in_=dram["nwb"][:, :])
    gmat_s = consts.tile([128, 8], F32R, tag="gmat", name="gmat_s")
    nc.sync.dma_start(out=gmat_s, in_=dram["gmat"][:, :])
    gbc_s = consts.tile([8, 128], F32R, tag="gbc", name="gbc_s")
    nc.sync.dma_start(out=gbc_s, in_=dram["gbc"][:, :])
    ones_row = consts.tile([1, 512], F32R, tag="ones_row", name="ones_row")
    nc.sync.dma_start(out=ones_row, in_=dram["onesr"][:, :])
    eps_t = consts.tile([8, 1], F32, tag="eps", name="eps_t")
    nc.vector.memset(eps_t, EPS)

    for b in range(B_LOC):
        # ---- load x (kept resident for the residual) --------------------
        xt = xb.tile([128, 4, T], F32, tag="xt", name=f"xt_b{b}")
        for j in range(4):
            nc.sync.dma_start(out=xt[:, j, :], in_=x_ap[b, 128 * j:128 * (j + 1), :])

        # ---- groupnorm stats: per-channel mean / E[x^2] -----------------
        mvs = gnp.tile([128, 8], F32R, tag="mvs", name=f"mvs_b{b}")
        for j in range(4):
            st6 = gnp.tile([128, 2, 6], F32, tag="st6", name=f"st6_b{b}_{j}")
            xr2 = xt[:, j, :].rearrange("p (n f) -> p n f", f=512)
            for sgi in range(2):
                nc.vector.bn_stats(out=st6[:, sgi, :], in_=xr2[:, sgi, :])
            mv = gnp.tile([128, 2], F32, tag="mv", name=f"mv_b{b}_{j}")
            nc.vector.bn_aggr(out=mv, in_=st6)
            # mvs[:, 2j] = mean_c ; mvs[:, 2j+1] = E[x^2]_c = var_c + mean_c^2
            nc.vector.tensor_copy(out=mvs[:, 2 * j:2 * j + 1], in_=mv[:, 0:1])
            sqm = gnp.tile([128, 1], F32, tag="sqm", name=f"sqm_b{b}_{j}")
            nc.vector.tensor_mul(out=sqm, in0=mv[:, 0:1], in1=mv[:, 0:1])
            nc.vector.tensor_add(out=mvs[:, 2 * j + 1:2 * j + 2], in0=mv[:, 1:2], in1=sqm)

        # ---- combine 16 channels per group via PE (cross-partition sum) -
        pg = msp.tile([128, 512], F32, tag="ms", name=f"pg_b{b}")
        for j in range(4):
            nc.tensor.matmul(pg[0:8, 2 * j:2 * j + 2], gmat_s,
                             mvs[:, 2 * j:2 * j + 2], start=True, stop=True)
        sg = gnp.tile([8, 8], F32, tag="sg", name=f"sg_b{b}")
        nc.vector.tensor_copy(out=sg, in_=pg[0:8, 0:8])
        sgr = sg.rearrange("p (j two) -> p j two", two=2)
        m2 = gnp.tile([8, 4], F32, tag="m2", name=f"m2_b{b}")
        nc.vector.tensor_mul(out=m2, in0=sgr[:, :, 0], in1=sgr[:, :, 0])
        varg = gnp.tile([8, 4], F32, tag="varg", name=f"varg_b{b}")
        nc.vector.tensor_sub(out=varg, in0=sgr[:, :, 1], in1=m2)
        # rstd = exp(-0.5 * ln(var + eps))  (Ln+Exp share one ACT table set)
        lnv = gnp.tile([8, 4], F32, tag="lnv", name=f"lnv_b{b}")
        nc.scalar.activation(out=lnv, in_=varg, func=AFT.Ln, bias=eps_t, scale=1.0)
        rstd = gnp.tile([8, 4], F32, tag="rstd", name=f"rstd_b{b}")
        nc.scalar.activation(out=rstd, in_=lnv, func=AFT.Exp, scale=-0.5)
        mr = gnp.tile([8, 8], F32R, tag="mr", name=f"mr_b{b}")
        mrr = mr.rearrange("p (j two) -> p j two", two=2)
        nc.vector.tensor_copy(out=mrr[:, :, 0], in_=sgr[:, :, 0])
        nc.vector.tensor_copy(out=mrr[:, :, 1], in_=rstd)

        # ---- normalize: xn = x*A + B per channel ------------------------
        xn = xnp.tile([128, 4, T], F32R, tag="xn", name=f"xn_b{b}")
        for j in range(4):
            pab = msp.tile([128, 512], F32, tag="ms", name=f"pab_b{b}_{j}")
            nc.tensor.matmul(pab[:, 0:2], gbc_s, mr[:, 2 * j:2 * j + 2],
                             start=True, stop=True)
            mc = gnp.tile([128, 2], F32, tag="mc", name=f"mc_b{b}_{j}")
            nc.vector.tensor_copy(out=mc, in_=pab[:, 0:2])
            Ac = gnp.tile([128, 1], F32, tag="Ac", name=f"Ac_b{b}_{j}")
            nc.vector.tensor_mul(out=Ac, in0=mc[:, 1:2], in1=nwb_s[:, j:j + 1])
            Bt = gnp.tile([128, 1], F32, tag="Bt", name=f"Bt_b{b}_{j}")
            nc.vector.tensor_mul(out=Bt, in0=mc[:, 0:1], in1=Ac)
            Bc = gnp.tile([128, 1], F32, tag="Bc", name=f"Bc_b{b}_{j}")
            nc.vector.tensor_sub(out=Bc, in0=nwb_s[:, 4 + j:5 + j], in1=Bt)
            nc.vector.tensor_scalar(out=xn[:, j, :], in0=xt[:, j, :],
                                    scalar1=Ac, scalar2=Bc,
                                    op0=ALU.mult, op1=ALU.add)

        # ---- V, transposed: vt[t, o_v] with interleaved ones columns ----
        vt = vtp.tile([128, 8, 520], BF16, tag="vt", name=f"vt_b{b}")
        vt4 = vt.rearrange("p s (h c) -> p s h c", c=65)
        nc.sync.dma_start(
            out=vt4[:, :, :, 64:65],
            in_=dram["vones"][:, :].rearrange("p (s h) -> p s h", h=8).unsqueeze(3))
        for tb in range(8):
            pv = msp.tile([128, 512], F32, tag="ms", name=f"pv_b{b}_{tb}")
            for j in range(4):
                nc.tensor.matmul(pv, xn[:, j, 128 * tb:128 * (tb + 1)],
                                 wq[j][:, 2 * C:3 * C],
                                 start=(j == 0), stop=(j == 3))
            nc.vector.tensor_add(
                out=vt4[:, tb, :, 0:64],
                in0=pv.rearrange("p (h c) -> p h c", c=64),
                in1=vb_s.rearrange("p (h c) -> p h c", c=64))

        # ---- Q/K produced just-in-time per head pair --------------------
        def make_qk(p):
            qt = qkp.tile([128, T], F32R, tag="qk", name=f"qt_b{b}_p{p}")
            kt = qkp.tile([128, T], F32R, tag="qk", name=f"kt_b{b}_p{p}")
            for dst, ocol, bcol in ((qt, 128 * p, p), (kt, C + 128 * p, 4 + p)):
                for th in range(2):
                    pq = msp.tile([128, 512], F32, tag="ms",
                                  name=f"pq_b{b}_p{p}_{bcol}_{th}")
                    for j in range(4):
                        nc.tensor.matmul(
                            pq, wq[j][:, ocol:ocol + 128],
                            xn[:, j, 512 * th:512 * (th + 1)],
                            start=(j == 0), stop=(j == 3))
                    nc.vector.tensor_scalar(
                        out=dst[:, 512 * th:512 * (th + 1)], in0=pq,
                        scalar1=qb_s[:, bcol:bcol + 1], scalar2=None, op0=ALU.add)
            return qt, kt

        a_sb = apool.tile([128, 4, T], F32R, tag="a", name=f"a_b{b}")
        qknext = make_qk(0)
        for p in range(4):
            qt, kt = qknext
            if p < 3:
                qknext = make_qk(p + 1)
            avv = [avp.tile([128, 512], F32, tag="av", name=f"av_b{b}_p{p}_{th}")
                   for th in range(2)]
            zps = zpp.tile([128, 512], F32, tag="z", name=f"z_b{b}_p{p}")
            for st in range(8):
                # scores grouped per t-half: [head A | head B] in one psum
                # tensor -> one exp per t-half, so the (0,0)-position AV
                # matmul of head A is the first consumer and absorbs the ACT
                # wait (col-offset fp32r matmuls cannot encode waits).
                sce = []
                for th in range(2):
                    sc = scp.tile([128, 1024], F32, tag="sc",
                                  name=f"sc_b{b}_p{p}_{st}_{th}")
                    nc.tensor.matmul(sc[:, 0:512],
                                     kt[0:64, 128 * st:128 * (st + 1)],
                                     qt[0:64, 512 * th:512 * (th + 1)],
                                     start=True, stop=True)
                    nc.tensor.matmul(sc[:, 512:1024],
                                     kt[64:128, 128 * st:128 * (st + 1)],
                                     qt[64:128, 512 * th:512 * (th + 1)],
                                     start=True, stop=True)
                    et = ep.tile([128, 1024], BF16, tag="e",
                                 name=f"e_b{b}_p{p}_{st}_{th}")
                    nc.scalar.activation(out=et, in_=sc, func=AFT.Exp, scale=SCALE2)
                    sce.append(et)
                first, last = (st == 0), (st == 7)
                for th in range(2):
                    eat = sce[th][:, 0:512]
                    ebt = sce[th][:, 512:1024]
                    nc.tensor.matmul(avv[th][0:64, :], vt4[:, st, 2 * p, 0:64],
                                     eat, start=first, stop=last,
                                     skip_group_check=True)
                    nc.tensor.matmul(zps[64 * th:64 * th + 1, :],
                                     vt4[:, st, 2 * p, 64:65],
                                     eat, start=first, stop=last,
                                     tile_position=(0, 64 * th),
                                     skip_group_check=True)
                    nc.tensor.matmul(avv[th][64:128, :], vt4[:, st, 2 * p + 1, 0:64],
                                     ebt, start=first, stop=last,
                                     skip_group_check=True)
                    nc.tensor.matmul(zps[64 * th + 32:64 * th + 33, :],
                                     vt4[:, st, 2 * p + 1, 64:65],
                                     ebt, start=first, stop=last,
                                     tile_position=(0, 64 * th + 32),
                                     skip_group_check=True)
            # softmax denominators -> reciprocal -> broadcast -> normalize
            rs = rpool.tile([128, 512], F32, tag="r", name=f"rs_b{b}_p{p}")
            for row in (0, 32, 64, 96):
                nc.vector.reciprocal(out=rs[row:row + 1, :], in_=zps[row:row + 1, :])
            for th in range(2):
                rb = rbp.tile([128, 512], F32, tag="rb", name=f"rb_b{b}_p{p}_{th}")
                srcA = rs[64 * th:64 * th + 1, None, :].broadcast_to([1, 64, 512])
                srcB = rs[64 * th + 32:64 * th + 33, None, :].broadcast_to([1, 64, 512])
                nc.sync.dma_start(out=rb[0:64, :], in_=srcA)
                nc.sync.dma_start(out=rb[64:128, :], in_=srcB)
                nc.vector.tensor_mul(out=a_sb[:, p, 512 * th:512 * (th + 1)],
                                     in0=avv[th], in1=rb)

        # ---- proj + residual -------------------------------------------
        for ot in range(4):
            for th in range(2):
                ph = msp.tile([128, 512], F32, tag="ms", name=f"ph_b{b}_{ot}_{th}")
                nc.tensor.matmul(ph, pb_s[0:1, 128 * ot:128 * (ot + 1)],
                                 ones_row, start=True, stop=False)
                for j in range(4):
                    nc.tensor.matmul(ph, wp[j][:, 128 * ot:128 * (ot + 1)],
                                     a_sb[:, j, 512 * th:512 * (th + 1)],
                                     start=False, stop=(j == 3))
                ob = opool.tile([128, 512], F32, tag="o", name=f"ob_b{b}_{ot}_{th}")
                nc.vector.tensor_add(out=ob, in0=xt[:, ot, 512 * th:512 * (th + 1)],
                                     in1=ph)
                nc.sync.dma_start(
                    out=out_ap[b, 128 * ot:128 * (ot + 1), 512 * th:512 * (th + 1)],
                    in_=ob)


def build_nc():
    nc = bacc.Bacc("TRN2", target_bir_lowering=False, debug=False)
    x_d = nc.declare_dram_parameter("x", [B_LOC, C, T], F32, isOutput=False)
    wqkvT_d = nc.declare_dram_parameter("wqkvT", [C, 3 * C], F32R, isOutput=False)
    wprojT_d = nc.declare_dram_parameter("wprojT", [C, C], F32R, isOutput=False)
    qb_d = nc.declare_dram_parameter("qb", [128, 8], F32, isOutput=False)
    vb_d = nc.declare_dram_parameter("vb", [128, 512], F32, isOutput=False)
    pb_d = nc.declare_dram_parameter("pb", [1, 512], F32R, isOutput=False)
    nwb_d = nc.declare_dram_parameter("nwb", [128, 8], F32, isOutput=False)
    gmat_d = nc.declare_dram_parameter("gmat", [128, 8], F32R, isOutput=False)
    gbc_d = nc.declare_dram_parameter("gbc", [8, 128], F32R, isOutput=False)
    onesr_d = nc.declare_dram_parameter("onesr", [1, 512], F32R, isOutput=False)
    vones_d = nc.declare_dram_parameter("vones", [128, 64], BF16, isOutput=False)
    out_d = nc.declare_dram_parameter("out", [B_LOC, C, T], F32, isOutput=True)
    dram = {
        "wqkvT": wqkvT_d[:, :], "wprojT": wprojT_d[:, :], "qb": qb_d,
        "vb": vb_d, "pb": pb_d, "nwb": nwb_d, "gmat": gmat_d, "gbc": gbc_d,
        "onesr": onesr_d, "vones": vones_d,
    }
    with tile.TileContext(nc) as tc:
        with (
            tc.tile_pool(name="consts", bufs=1) as consts,
            tc.tile_pool(name="xb", bufs=2) as xb,
            tc.tile_pool(name="xnp", bufs=2) as xnp,
            tc.tile_pool(name="vtp", bufs=2) as vtp,
            tc.tile_pool(name="qkp", bufs=4) as qkp,
            tc.tile_pool(name="ep", bufs=3) as ep,
            tc.tile_pool(name="apool", bufs=1) as apool,
            tc.tile_pool(name="opool", bufs=2) as opool,
            tc.tile_pool(name="rpool", bufs=2) as rpool,
            tc.tile_pool(name="rbp", bufs=3) as rbp,
            tc.tile_pool(name="gnp", bufs=2) as gnp,
            tc.tile_pool(name="scp", bufs=2, space="PSUM") as scp,
            tc.tile_pool(name="avp", bufs=2, space="PSUM") as avp,
            tc.tile_pool(name="zpp", bufs=1, space="PSUM") as zpp,
            tc.tile_pool(name="msp", bufs=1, space="PSUM") as msp,
        ):
            pools = (consts, xb, xnp, vtp, qkp, ep, apool, opool, rpool, rbp,
                     gnp, scp, avp, zpp, msp)
            _emit(nc, tc, pools, x_d[:, :, :], out_d[:, :, :], dram)
    nc.finalize()
    return nc


_CACHE = {}


def _host_inputs(x, norm_w, norm_b, qkv_w, qkv_b, proj_w, proj_b):
    f = lambda a: np.ascontiguousarray(np.asarray(a, dtype=np.float32))
    x = f(x).reshape(B_TOT, C, T)
    qkv_w, qkv_b, proj_w, proj_b = f(qkv_w), f(qkv_b), f(proj_w), f(proj_b)
    norm_w, norm_b = f(norm_w), f(norm_b)
    shared = {
        "wqkvT": np.ascontiguousarray(qkv_w.T),
        "wprojT": np.ascontiguousarray(proj_w.T),
        "qb": np.ascontiguousarray(qkv_b[:1024].reshape(8, 128).T),
        "vb": np.ascontiguousarray(np.broadcast_to(qkv_b[1024:], (128, 512))),
        "pb": np.ascontiguousarray(proj_b.reshape(1, 512)),
        "nwb": np.ascontiguousarray(np.concatenate(
            [norm_w.reshape(4, 128).T, norm_b.reshape(4, 128).T], axis=1)),
    }
    p = np.arange(128)
    gmat = np.zeros((128, 8), np.float32)
    gmat[p, p // 16] = 1.0 / 16.0
    gbc = np.zeros((8, 128), np.float32)
    gbc[p // 16, p] = 1.0
    shared["gmat"] = gmat
    shared["gbc"] = gbc
    shared["onesr"] = np.ones((1, 512), np.float32)
    shared["vones"] = np.ones((128, 64), ml_dtypes.bfloat16)
    in_maps = []
    for i in range(N_CORES):
        m = dict(shared)
        m["x"] = np.ascontiguousarray(x[B_LOC * i:B_LOC * (i + 1)])
        in_maps.append(m)
    return in_maps


def kernel(x, norm_w, norm_b, qkv_w, qkv_b, proj_w, proj_b):
    from concourse.bass_utils import run_bass_kernel_spmd

    if "nc" not in _CACHE:
        _CACHE["nc"] = build_nc()
    in_maps = _host_inputs(x, norm_w, norm_b, qkv_w, qkv_b, proj_w, proj_b)
    res = run_bass_kernel_spmd(_CACHE["nc"], in_maps, core_ids=list(range(N_CORES)))
    out = np.concatenate([r["out"] for r in res.results], axis=0)
    return np.ascontiguousarray(out.reshape(B_TOT, C, 32, 32).astype(np.float32))


if __name__ == "__main__":
    nc = build_nc()
    print("built ok; instructions:",
          sum(len(bb.instructions) for f in nc.m.functions for bb in f.basicblocks)
          if hasattr(nc.m.functions[0], "basicblocks") else "?")


# revision 27
# speedup vs baseline: 163.0204x; 163.0204x over previous
"""AttentionBlock (GroupNorm + single-head-dim-64 MHA + proj + residual) on 8 trn2 cores.

Data-parallel over batch: 16 batches -> 2 per core. Params replicated.
All matmuls in fp32r (full PE rate at N=512, ~tf32 precision), softmax in fp32.
"""

import ml_dtypes
import numpy as np

import concourse.bass as bass
import concourse.bacc as bacc
import concourse.mybir as mybir
import concourse.tile as tile

F32 = mybir.dt.float32
F32R = mybir.dt.float32r
BF16 = mybir.dt.bfloat16
ALU = mybir.AluOpType
AFT = mybir.ActivationFunctionType

B_TOT, C, T = 16, 512, 1024  # T = H*W = 32*32
N_CORES = 8
B_LOC = B_TOT // N_CORES  # 2 batches per core
NG = 32  # groups
NH = 8  # heads
CH = 64  # head channels
EPS = 1e-5
SCALE2 = 0.125  # (1/sqrt(sqrt(64)))**2 applied to q@k scores


def _emit(nc, tc, pools, x_ap, out_ap, dram, reps=1, mode='full', loop_iters=0):
    (consts, xb, xnp, vtp, qkp, ep, apool, opool, rpool, rbp, gnp,
     scp, avp, msp, dramp) = pools

    # ---- load constants -------------------------------------------------
    wq = []
    for j in range(4):
        wqt = consts.tile([128, 3 * C], F32R, tag=f"wq{j}", name=f"wq{j}")
        nc.sync.dma_start(out=wqt, in_=dram["wqkvT"][128 * j:128 * (j + 1), :])
        wq.append(wqt)
    wp = []
    for j in range(4):
        wpt = consts.tile([128, C], F32R, tag=f"wp{j}", name=f"wp{j}")
        nc.sync.dma_start(out=wpt, in_=dram["wprojT"][128 * j:128 * (j + 1), :])
        wp.append(wpt)
    qb_s = consts.tile([128, 8], F32, tag="qb", name="qb_s")
    nc.sync.dma_start(out=qb_s, in_=dram["qb"][:, :])
    vb_s = consts.tile([128, 512], F32, tag="vb", name="vb_s")
    nc.sync.dma_start(out=vb_s, in_=dram["vb"][:, :])
    pb_s = consts.tile([1, 512], F32R, tag="pb", name="pb_s")
    nc.sync.dma_start(out=pb_s, in_=dram["pb"][:, :])
    nwb_s = consts.tile([128, 8], F32, tag="nwb", name="nwb_s")
    nc.sync.dma_start(out=nwb_s, in_=dram["nwb"][:, :])
    gmat_s = consts.tile([128, 8], F32R, tag="gmat", name="gmat_s")
    nc.sync.dma_start(out=gmat_s, in_=dram["gmat"][:, :])
    gbc_s = consts.tile([8, 128], F32R, tag="gbc", name="gbc_s")
    nc.sync.dma_start(out=gbc_s, in_=dram["gbc"][:, :])
    ones_row = consts.tile([1, 512], F32R, tag="ones_row", name="ones_row")
    nc.sync.dma_start(out=ones_row, in_=dram["onesr"][:, :])
    eps_t = consts.tile([8, 1], F32, tag="eps", name="eps_t")
    nc.vector.memset(eps_t, EPS)

    import contextlib
    loop_cm = tc.For_i(0, loop_iters, 1) if loop_iters else contextlib.nullcontext()
    with loop_cm:
      for rep in range(reps):
       for b0 in range(B_LOC):
        b = f"{rep}_{b0}" if reps > 1 else str(b0)
        # ---- load x (kept resident for the residual) --------------------
        xt = xb.tile([128, 4, T], F32, tag="xt", name=f"xt_b{b}")
        for j in range(4):
            nc.sync.dma_start(out=xt[:, j, :], in_=x_ap[b0, 128 * j:128 * (j + 1), :])

        if mode == 'io':
            for ot in range(4):
                obio = opool.tile([128, 1024], F32, tag="oio", name=f"obio_{b}_{ot}")
                nc.vector.tensor_copy(out=obio, in_=xt[:, ot, :])
                nc.sync.dma_start(out=out_ap[b0, 128 * ot:128 * (ot + 1), :], in_=obio)
            continue
        # ---- groupnorm stats: per-channel mean / E[x^2] -----------------
        mvs = gnp.tile([128, 8], F32R, tag="mvs", name=f"mvs_b{b}")
        for j in range(4):
            st6 = gnp.tile([128, 2, 6], F32, tag="st6", name=f"st6_b{b}_{j}")
            xr2 = xt[:, j, :].rearrange("p (n f) -> p n f", f=512)
            for sgi in range(2):
                nc.vector.bn_stats(out=st6[:, sgi, :], in_=xr2[:, sgi, :])
            mv = gnp.tile([128, 2], F32, tag="mv", name=f"mv_b{b}_{j}")
            nc.vector.bn_aggr(out=mv, in_=st6)
            # mvs[:, 2j] = mean_c ; mvs[:, 2j+1] = E[x^2]_c = var_c + mean_c^2
            nc.vector.tensor_copy(out=mvs[:, 2 * j:2 * j + 1], in_=mv[:, 0:1])
            sqm = gnp.tile([128, 1], F32, tag="sqm", name=f"sqm_b{b}_{j}")
            nc.vector.tensor_mul(out=sqm, in0=mv[:, 0:1], in1=mv[:, 0:1])
            nc.vector.tensor_add(out=mvs[:, 2 * j + 1:2 * j + 2], in0=mv[:, 1:2], in1=sqm)

        # ---- combine 16 channels per group via PE (cross-partition sum) -
        pg = msp.tile([128, 512], F32, tag="ms", name=f"pg_b{b}")
        for j in range(4):
            nc.tensor.matmul(pg[0:8, 2 * j:2 * j + 2], gmat_s,
                             mvs[:, 2 * j:2 * j + 2], start=True, stop=True)
        sg = gnp.tile([8, 8], F32, tag="sg", name=f"sg_b{b}")
        nc.vector.tensor_copy(out=sg, in_=pg[0:8, 0:8])
        sgr = sg.rearrange("p (j two) -> p j two", two=2)
        m2 = gnp.tile([8, 4], F32, tag="m2", name=f"m2_b{b}")
        nc.vector.tensor_mul(out=m2, in0=sgr[:, :, 0], in1=sgr[:, :, 0])
        varg = gnp.tile([8, 4], F32, tag="varg", name=f"varg_b{b}")
        nc.vector.tensor_sub(out=varg, in0=sgr[:, :, 1], in1=m2)
        # rstd = exp(-0.5 * ln(var + eps))  (Ln+Exp share one ACT table set)
        lnv = gnp.tile([8, 4], F32, tag="lnv", name=f"lnv_b{b}")
        nc.scalar.activation(out=lnv, in_=varg, func=AFT.Ln, bias=eps_t, scale=1.0)
        rstd = gnp.tile([8, 4], F32, tag="rstd", name=f"rstd_b{b}")
        nc.scalar.activation(out=rstd, in_=lnv, func=AFT.Exp, scale=-0.5)
        mr = gnp.tile([8, 8], F32R, tag="mr", name=f"mr_b{b}")
        mrr = mr.rearrange("p (j two) -> p j two", two=2)
        nc.vector.tensor_copy(out=mrr[:, :, 0], in_=sgr[:, :, 0])
        nc.vector.tensor_copy(out=mrr[:, :, 1], in_=rstd)

        # ---- normalize: xn = x*A + B per channel ------------------------
        xn = xnp.tile([128, 4, T], F32R, tag="xn", name=f"xn_b{b}")
        for j in range(4):
            pab = msp.tile([128, 512], F32, tag="ms", name=f"pab_b{b}_{j}")
            nc.tensor.matmul(pab[:, 0:2], gbc_s, mr[:, 2 * j:2 * j + 2],
                             start=True, stop=True)
            mc = gnp.tile([128, 2], F32, tag="mc", name=f"mc_b{b}_{j}")
            nc.vector.tensor_copy(out=mc, in_=pab[:, 0:2])
            Ac = gnp.tile([128, 1], F32, tag="Ac", name=f"Ac_b{b}_{j}")
            nc.vector.tensor_mul(out=Ac, in0=mc[:, 1:2], in1=nwb_s[:, j:j + 1])
            Bt = gnp.tile([128, 1], F32, tag="Bt", name=f"Bt_b{b}_{j}")
            nc.vector.tensor_mul(out=Bt, in0=mc[:, 0:1], in1=Ac)
            Bc = gnp.tile([128, 1], F32, tag="Bc", name=f"Bc_b{b}_{j}")
            nc.vector.tensor_sub(out=Bc, in0=nwb_s[:, 4 + j:5 + j], in1=Bt)
            nc.vector.tensor_scalar(out=xn[:, j, :], in0=xt[:, j, :],
                                    scalar1=Ac, scalar2=Bc,
                                    op0=ALU.mult, op1=ALU.add)

        # ---- V, transposed: vt[t, o_v] with interleaved ones columns ----
        vt = vtp.tile([128, 8, 520], BF16, tag="vt", name=f"vt_b{b}")
        vt4 = vt.rearrange("p s (h c) -> p s h c", c=65)
        nc.sync.dma_start(
            out=vt4[:, :, :, 64:65],
            in_=dram["vones"][:, :].rearrange("p (s h) -> p s h", h=8).unsqueeze(3))
        for tb in range(8):
            pv = msp.tile([128, 512], F32, tag="ms", name=f"pv_b{b}_{tb}")
            for j in range(4):
                nc.tensor.matmul(pv, xn[:, j, 128 * tb:128 * (tb + 1)],
                                 wq[j][:, 2 * C:3 * C],
                                 start=(j == 0), stop=(j == 3))
            nc.vector.tensor_add(
                out=vt4[:, tb, :, 0:64],
                in0=pv.rearrange("p (h c) -> p h c", c=64),
                in1=vb_s.rearrange("p (h c) -> p h c", c=64))

        # ---- Q/K produced just-in-time per head pair --------------------
        def make_qk(p):
            qt = qkp.tile([128, T], F32R, tag="qk", name=f"qt_b{b}_p{p}")
            kt = qkp.tile([128, T], F32R, tag="qk", name=f"kt_b{b}_p{p}")
            for dst, ocol, bcol in ((qt, 128 * p, p), (kt, C + 128 * p, 4 + p)):
                for th in range(2):
                    pq = msp.tile([128, 512], F32, tag="ms",
                                  name=f"pq_b{b}_p{p}_{bcol}_{th}")
                    for j in range(4):
                        nc.tensor.matmul(
                            pq, wq[j][:, ocol:ocol + 128],
                            xn[:, j, 512 * th:512 * (th + 1)],
                            start=(j == 0), stop=(j == 3))
                    nc.vector.tensor_scalar(
                        out=dst[:, 512 * th:512 * (th + 1)], in0=pq,
                        scalar1=qb_s[:, bcol:bcol + 1], scalar2=None, op0=ALU.add)
            return qt, kt

        a_sb = apool.tile([128, 4, T], F32R, tag="a", name=f"a_b{b}")
        if mode == 'noattn':
            pairs = []
        else:
            pairs = range(4)
        qknext = make_qk(0) if mode != 'noattn' else None
        for p in pairs:
            qt, kt = qknext
            if p < 3:
                qknext = make_qk(p + 1)
            skip_av = (mode == 'noav')
            # one [65, 512] accumulator per (th, head): rows 0-63 = a,
            # row 64 = softmax denominator (from vt's interleaved ones col)
            avv = [[avp.tile([65, 512], F32, tag="av",
                             name=f"av_b{b}_p{p}_{th}_{hh}")
                    for hh in range(2)] for th in range(2)]
            for st in range(8):
                sce = []
                for th in range(2):
                    ets = []
                    for hh in range(2):
                        sc = scp.tile([128, 512], F32, tag="sc",
                                      name=f"sc_b{b}_p{p}_{st}_{th}_{hh}")
                        nc.tensor.matmul(sc,
                                         kt[64 * hh:64 * hh + 64,
                                            128 * st:128 * (st + 1)],
                                         qt[64 * hh:64 * hh + 64,
                                            512 * th:512 * (th + 1)],
                                         start=True, stop=True)
                        et = ep.tile([128, 512], BF16, tag="e",
                                     name=f"e_b{b}_p{p}_{st}_{th}_{hh}")
                        nc.scalar.activation(out=et, in_=sc, func=AFT.Exp,
                                             scale=SCALE2)
                        ets.append(et)
                    sce.append(ets)
                first, last = (st == 0), (st == 7)
                if skip_av:
                    continue
                for th in range(2):
                    for hh in range(2):
                        nc.tensor.matmul(avv[th][hh],
                                         vt4[:, st, 2 * p + hh, 0:65],
                                         sce[th][hh][:, :],
                                         start=first, stop=last)
            # softmax denominators -> reciprocal -> broadcast -> normalize
            if skip_av:
                continue
            # stage accumulators to SBUF right away so the PSUM banks free
            # up for the next pair; normalize entirely from SBUF afterwards.
            aw = []
            for th in range(2):
                awh = []
                for hh in range(2):
                    a1 = rpool.tile([65, 512], F32, bufs=8, tag="aw",
                                    name=f"aw_b{b}_p{p}_{th}_{hh}")
                    nc.vector.tensor_copy(out=a1, in_=avv[th][hh])
                    awh.append(a1)
                aw.append(awh)
            # gather the 4 Z rows (partition 64 of each), spread across
            # partitions via DMA so the iterative DVE reciprocal is
            # partition-parallel instead of free-dim-serial.
            zr2 = rpool.tile([16, 128], F32, tag="zr2", name=f"zr2_b{b}_p{p}")
            for th in range(2):
                for hh in range(2):
                    k = 2 * th + hh
                    nc.sync.dma_start(
                        out=zr2[4 * k:4 * k + 4, :],
                        in_=aw[th][hh][64:65, :].rearrange("p (a c) -> p a c", a=4))
            r2 = rpool.tile([16, 128], F32, tag="r2", name=f"r2_b{b}_p{p}")
            nc.vector.reciprocal(out=r2, in_=zr2)
            rbd = dramp.tile([4, 512], F32, tag="rbd", name=f"rbd_b{b}_p{p}")
            nc.sync.dma_start(out=rbd, in_=r2)
            for th in range(2):
                rb = rbp.tile([64, 1024], F32, tag="rb", name=f"rb_b{b}_p{p}_{th}")
                nc.sync.dma_start(out=rb[:, 0:512],
                                  in_=rbd[2 * th:2 * th + 1, :].partition_broadcast(64))
                nc.sync.dma_start(out=rb[:, 512:1024],
                                  in_=rbd[2 * th + 1:2 * th + 2, :].partition_broadcast(64))
                nc.vector.tensor_mul(out=a_sb[0:64, p, 512 * th:512 * (th + 1)],
                                     in0=aw[th][0][0:64, :], in1=rb[:, 0:512])
                tb2 = rbp.tile([64, 512], F32R, tag="tb2", name=f"tb2_b{b}_p{p}_{th}")
                nc.vector.tensor_mul(out=tb2, in0=aw[th][1][0:64, :],
                                     in1=rb[:, 512:1024])
                nc.sync.dma_start(out=a_sb[64:128, p, 512 * th:512 * (th + 1)],
                                  in_=tb2)

        # ---- proj + residual -------------------------------------------
        for ot in range(4):
            for th in range(2):
                ph = msp.tile([128, 512], F32, tag="ms", name=f"ph_b{b}_{ot}_{th}")
                nc.tensor.matmul(ph, pb_s[0:1, 128 * ot:128 * (ot + 1)],
                                 ones_row, start=True, stop=False)
                rhs_src = a_sb if mode == 'full' else xn
                for j in range(4):
                    nc.tensor.matmul(ph, wp[j][:, 128 * ot:128 * (ot + 1)],
                                     rhs_src[:, j, 512 * th:512 * (th + 1)],
                                     start=False, stop=(j == 3))
                ob = opool.tile([128, 512], F32, tag="o", name=f"ob_b{b}_{ot}_{th}")
                nc.vector.tensor_add(out=ob, in0=xt[:, ot, 512 * th:512 * (th + 1)],
                                     in1=ph)
                nc.sync.dma_start(
                    out=out_ap[b0, 128 * ot:128 * (ot + 1), 512 * th:512 * (th + 1)],
                    in_=ob)


def build_nc(reps=1, mode='full', loop_iters=0):
    nc = bacc.Bacc("TRN2", target_bir_lowering=False, debug=False)
    x_d = nc.declare_dram_parameter("x", [B_LOC, C, T], F32, isOutput=False)
    wqkvT_d = nc.declare_dram_parameter("wqkvT", [C, 3 * C], F32R, isOutput=False)
    wprojT_d = nc.declare_dram_parameter("wprojT", [C, C], F32R, isOutput=False)
    qb_d = nc.declare_dram_parameter("qb", [128, 8], F32, isOutput=False)
    vb_d = nc.declare_dram_parameter("vb", [128, 512], F32, isOutput=False)
    pb_d = nc.declare_dram_parameter("pb", [1, 512], F32R, isOutput=False)
    nwb_d = nc.declare_dram_parameter("nwb", [128, 8], F32, isOutput=False)
    gmat_d = nc.declare_dram_parameter("gmat", [128, 8], F32R, isOutput=False)
    gbc_d = nc.declare_dram_parameter("gbc", [8, 128], F32R, isOutput=False)
    onesr_d = nc.declare_dram_parameter("onesr", [1, 512], F32R, isOutput=False)
    vones_d = nc.declare_dram_parameter("vones", [128, 64], BF16, isOutput=False)
    out_d = nc.declare_dram_parameter("out", [B_LOC, C, T], F32, isOutput=True)
    dram = {
        "wqkvT": wqkvT_d[:, :], "wprojT": wprojT_d[:, :], "qb": qb_d,
        "vb": vb_d, "pb": pb_d, "nwb": nwb_d, "gmat": gmat_d, "gbc": gbc_d,
        "onesr": onesr_d, "vones": vones_d,
    }
    with tile.TileContext(nc) as tc:
        with (
            tc.tile_pool(name="consts", bufs=1) as consts,
            tc.tile_pool(name="xb", bufs=2) as xb,
            tc.tile_pool(name="xnp", bufs=2) as xnp,
            tc.tile_pool(name="vtp", bufs=2) as vtp,
            tc.tile_pool(name="qkp", bufs=4) as qkp,
            tc.tile_pool(name="ep", bufs=3) as ep,
            tc.tile_pool(name="apool", bufs=1) as apool,
            tc.tile_pool(name="opool", bufs=2) as opool,
            tc.tile_pool(name="rpool", bufs=2) as rpool,
            tc.tile_pool(name="rbp", bufs=3) as rbp,
            tc.tile_pool(name="gnp", bufs=2) as gnp,
            tc.tile_pool(name="scp", bufs=3, space="PSUM") as scp,
            tc.tile_pool(name="avp", bufs=4, space="PSUM") as avp,
            tc.tile_pool(name="msp", bufs=1, space="PSUM") as msp,
            tc.tile_pool(name="dramp", bufs=4, space="DRAM") as dramp,
        ):
            pools = (consts, xb, xnp, vtp, qkp, ep, apool, opool, rpool, rbp,
                     gnp, scp, avp, msp, dramp)
            _emit(nc, tc, pools, x_d[:, :, :], out_d[:, :, :], dram, reps=reps, mode=mode, loop_iters=loop_iters)
    nc.finalize()
    return nc


_CACHE = {}


def _host_inputs(x, norm_w, norm_b, qkv_w, qkv_b, proj_w, proj_b):
    f = lambda a: np.ascontiguousarray(np.asarray(a, dtype=np.float32))
    x = f(x).reshape(B_TOT, C, T)
    qkv_w, qkv_b, proj_w, proj_b = f(qkv_w), f(qkv_b), f(proj_w), f(proj_b)
    norm_w, norm_b = f(norm_w), f(norm_b)
    shared = {
        "wqkvT": np.ascontiguousarray(qkv_w.T),
        "wprojT": np.ascontiguousarray(proj_w.T),
        "qb": np.ascontiguousarray(qkv_b[:1024].reshape(8, 128).T),
        "vb": np.ascontiguousarray(np.broadcast_to(qkv_b[1024:], (128, 512))),
        "pb": np.ascontiguousarray(proj_b.reshape(1, 512)),
        "nwb": np.ascontiguousarray(np.concatenate(
            [norm_w.reshape(4, 128).T, norm_b.reshape(4, 128).T], axis=1)),
    }
    p = np.arange(128)
    gmat = np.zeros((128, 8), np.float32)
    gmat[p, p // 16] = 1.0 / 16.0
    gbc = np.zeros((8, 128), np.float32)
    gbc[p // 16, p] = 1.0
    shared["gmat"] = gmat
    shared["gbc"] = gbc
    shared["onesr"] = np.ones((1, 512), np.float32)
    shared["vones"] = np.ones((128, 64), ml_dtypes.bfloat16)
    in_maps = []
    for i in range(N_CORES):
        m = dict(shared)
        m["x"] = np.ascontiguousarray(x[B_LOC * i:B_LOC * (i + 1)])
        in_maps.append(m)
    return in_maps


def kernel(x, norm_w, norm_b, qkv_w, qkv_b, proj_w, proj_b):
    from concourse.bass_utils import run_bass_kernel_spmd

    if "nc" not in _CACHE:
        _CACHE["nc"] = build_nc()
    in_maps = _host_inputs(x, norm_w, norm_b, qkv_w, qkv_b, proj_w, proj_b)
    res = run_bass_kernel_spmd(_CACHE["nc"], in_maps, core_ids=list(range(N_CORES)))
    out = np.concatenate([r["out"] for r in res.results], axis=0)
    return np.ascontiguousarray(out.reshape(B_TOT, C, 32, 32).astype(np.float32))


if __name__ == "__main__":
    nc = build_nc()
    print("built ok; instructions:",
          sum(len(bb.instructions) for f in nc.m.functions for bb in f.basicblocks)
          if hasattr(nc.m.functions[0], "basicblocks") else "?")


# revision 28
# speedup vs baseline: 402.9329x; 2.4717x over previous
"""AttentionBlock (GroupNorm + single-head-dim-64 MHA + proj + residual) on 8 trn2 cores.

Data-parallel over batch: 16 batches -> 2 per core. Params replicated.
All matmuls in fp32r (full PE rate at N=512, ~tf32 precision), softmax in fp32.
"""

import ml_dtypes
import numpy as np

import concourse.bass as bass
import concourse.bacc as bacc
import concourse.mybir as mybir
import concourse.tile as tile

F32 = mybir.dt.float32
F32R = mybir.dt.float32r
BF16 = mybir.dt.bfloat16
ALU = mybir.AluOpType
AFT = mybir.ActivationFunctionType

B_TOT, C, T = 16, 512, 1024  # T = H*W = 32*32
N_CORES = 8
B_LOC = B_TOT // N_CORES  # 2 batches per core
NG = 32  # groups
NH = 8  # heads
CH = 64  # head channels
EPS = 1e-5
SCALE2 = 0.125  # (1/sqrt(sqrt(64)))**2 applied to q@k scores


def _emit(nc, tc, pools, x_ap, out_ap, dram, reps=1, mode='full', loop_iters=0):
    (consts, xb, xnp, vtp, qkp, ep, apool, opool, rpool, rbp, gnp,
     scp, avp, msp, dramp) = pools

    # ---- load constants -------------------------------------------------
    wq = []
    for j in range(4):
        wqt = consts.tile([128, 3 * C], F32R, tag=f"wq{j}", name=f"wq{j}")
        nc.sync.dma_start(out=wqt, in_=dram["wqkvT"][128 * j:128 * (j + 1), :])
        wq.append(wqt)
    wp = []
    for j in range(4):
        wpt = consts.tile([128, C], F32R, tag=f"wp{j}", name=f"wp{j}")
        nc.sync.dma_start(out=wpt, in_=dram["wprojT"][128 * j:128 * (j + 1), :])
        wp.append(wpt)
    qb_s = consts.tile([128, 8], F32, tag="qb", name="qb_s")
    nc.sync.dma_start(out=qb_s, in_=dram["qb"][:, :])
    vb_s = consts.tile([128, 512], F32, tag="vb", name="vb_s")
    nc.sync.dma_start(out=vb_s, in_=dram["vb"][:, :])
    pb_s = consts.tile([1, 512], F32R, tag="pb", name="pb_s")
    nc.sync.dma_start(out=pb_s, in_=dram["pb"][:, :])
    nwb_s = consts.tile([128, 8], F32, tag="nwb", name="nwb_s")
    nc.sync.dma_start(out=nwb_s, in_=dram["nwb"][:, :])
    gmat_s = consts.tile([128, 8], F32R, tag="gmat", name="gmat_s")
    nc.sync.dma_start(out=gmat_s, in_=dram["gmat"][:, :])
    gbc_s = consts.tile([8, 128], F32R, tag="gbc", name="gbc_s")
    nc.sync.dma_start(out=gbc_s, in_=dram["gbc"][:, :])
    ones_row = consts.tile([1, 512], F32R, tag="ones_row", name="ones_row")
    nc.sync.dma_start(out=ones_row, in_=dram["onesr"][:, :])
    eps_t = consts.tile([8, 1], F32, tag="eps", name="eps_t")
    nc.vector.memset(eps_t, EPS)

    import contextlib
    loop_cm = tc.For_i(0, loop_iters, 1) if loop_iters else contextlib.nullcontext()
    with loop_cm:
      for rep in range(reps):
       for b0 in range(B_LOC):
        b = f"{rep}_{b0}" if reps > 1 else str(b0)
        # ---- load x (kept resident for the residual) --------------------
        xt = []
        for j in range(4):
            x1 = xb.tile([128, T], F32, tag="xt", bufs=8, name=f"xt_b{b}_{j}")
            nc.sync.dma_start(out=x1, in_=x_ap[b0, 128 * j:128 * (j + 1), :])
            xt.append(x1)

        if mode == 'io':
            for ot in range(4):
                obio = opool.tile([128, 1024], F32, tag="oio", name=f"obio_{b}_{ot}")
                nc.vector.tensor_copy(out=obio, in_=xt[ot])
                nc.sync.dma_start(out=out_ap[b0, 128 * ot:128 * (ot + 1), :], in_=obio)
            continue
        # ---- groupnorm stats: per-channel mean / E[x^2] -----------------
        mvs = gnp.tile([128, 8], F32R, tag="mvs", name=f"mvs_b{b}")
        for j in range(4):
            st6 = gnp.tile([128, 2, 6], F32, tag="st6", name=f"st6_b{b}_{j}")
            xr2 = xt[j].rearrange("p (n f) -> p n f", f=512)
            for sgi in range(2):
                nc.vector.bn_stats(out=st6[:, sgi, :], in_=xr2[:, sgi, :])
            mv = gnp.tile([128, 2], F32, tag="mv", name=f"mv_b{b}_{j}")
            nc.vector.bn_aggr(out=mv, in_=st6)
            # mvs[:, 2j] = mean_c ; mvs[:, 2j+1] = E[x^2]_c = var_c + mean_c^2
            nc.vector.tensor_copy(out=mvs[:, 2 * j:2 * j + 1], in_=mv[:, 0:1])
            sqm = gnp.tile([128, 1], F32, tag="sqm", name=f"sqm_b{b}_{j}")
            nc.vector.tensor_mul(out=sqm, in0=mv[:, 0:1], in1=mv[:, 0:1])
            nc.vector.tensor_add(out=mvs[:, 2 * j + 1:2 * j + 2], in0=mv[:, 1:2], in1=sqm)

        # ---- combine 16 channels per group via PE (cross-partition sum) -
        pg = msp.tile([128, 512], F32, tag="ms", name=f"pg_b{b}")
        for j in range(4):
            nc.tensor.matmul(pg[0:8, 2 * j:2 * j + 2], gmat_s,
                             mvs[:, 2 * j:2 * j + 2], start=True, stop=True)
        sg = gnp.tile([8, 8], F32, tag="sg", name=f"sg_b{b}")
        nc.vector.tensor_copy(out=sg, in_=pg[0:8, 0:8])
        sgr = sg.rearrange("p (j two) -> p j two", two=2)
        m2 = gnp.tile([8, 4], F32, tag="m2", name=f"m2_b{b}")
        nc.vector.tensor_mul(out=m2, in0=sgr[:, :, 0], in1=sgr[:, :, 0])
        varg = gnp.tile([8, 4], F32, tag="varg", name=f"varg_b{b}")
        nc.vector.tensor_sub(out=varg, in0=sgr[:, :, 1], in1=m2)
        # rstd = exp(-0.5 * ln(var + eps))  (Ln+Exp share one ACT table set)
        lnv = gnp.tile([8, 4], F32, tag="lnv", name=f"lnv_b{b}")
        nc.scalar.activation(out=lnv, in_=varg, func=AFT.Ln, bias=eps_t, scale=1.0)
        rstd = gnp.tile([8, 4], F32, tag="rstd", name=f"rstd_b{b}")
        nc.scalar.activation(out=rstd, in_=lnv, func=AFT.Exp, scale=-0.5)
        mr = gnp.tile([8, 8], F32R, tag="mr", name=f"mr_b{b}")
        mrr = mr.rearrange("p (j two) -> p j two", two=2)
        nc.vector.tensor_copy(out=mrr[:, :, 0], in_=sgr[:, :, 0])
        nc.vector.tensor_copy(out=mrr[:, :, 1], in_=rstd)

        # ---- normalize: xn = x*A + B per channel ------------------------
        xn = []
        for j in range(4):
            pab = msp.tile([128, 512], F32, tag="ms", name=f"pab_b{b}_{j}")
            nc.tensor.matmul(pab[:, 0:2], gbc_s, mr[:, 2 * j:2 * j + 2],
                             start=True, stop=True)
            mc = gnp.tile([128, 2], F32, tag="mc", name=f"mc_b{b}_{j}")
            nc.vector.tensor_copy(out=mc, in_=pab[:, 0:2])
            Ac = gnp.tile([128, 1], F32, tag="Ac", name=f"Ac_b{b}_{j}")
            nc.vector.tensor_mul(out=Ac, in0=mc[:, 1:2], in1=nwb_s[:, j:j + 1])
            Bt = gnp.tile([128, 1], F32, tag="Bt", name=f"Bt_b{b}_{j}")
            nc.vector.tensor_mul(out=Bt, in0=mc[:, 0:1], in1=Ac)
            Bc = gnp.tile([128, 1], F32, tag="Bc", name=f"Bc_b{b}_{j}")
            nc.vector.tensor_sub(out=Bc, in0=nwb_s[:, 4 + j:5 + j], in1=Bt)
            xn1 = xnp.tile([128, T], F32R, tag="xn", bufs=8, name=f"xn_b{b}_{j}")
            nc.vector.tensor_scalar(out=xn1, in0=xt[j],
                                    scalar1=Ac, scalar2=Bc,
                                    op0=ALU.mult, op1=ALU.add)
            xn.append(xn1)

        # ---- V, transposed: vt[t, o_v] with interleaved ones columns ----
        vt = vtp.tile([128, 8, 520], BF16, tag="vt", name=f"vt_b{b}")
        vt4 = vt.rearrange("p s (h c) -> p s h c", c=65)
        nc.sync.dma_start(
            out=vt4[:, :, :, 64:65],
            in_=dram["vones"][:, :].rearrange("p (s h) -> p s h", h=8).unsqueeze(3))
        for tb in range(8):
            pv = msp.tile([128, 512], F32, tag="ms", name=f"pv_b{b}_{tb}")
            for j in range(4):
                nc.tensor.matmul(pv, xn[j][:, 128 * tb:128 * (tb + 1)],
                                 wq[j][:, 2 * C:3 * C],
                                 start=(j == 0), stop=(j == 3))
            nc.vector.tensor_add(
                out=vt4[:, tb, :, 0:64],
                in0=pv.rearrange("p (h c) -> p h c", c=64),
                in1=vb_s.rearrange("p (h c) -> p h c", c=64))

        # ---- Q/K produced just-in-time per head pair --------------------
        def make_qk(p):
            qt = qkp.tile([128, T], F32R, tag="qk", name=f"qt_b{b}_p{p}")
            kt = qkp.tile([128, T], F32R, tag="qk", name=f"kt_b{b}_p{p}")
            for dst, ocol, bcol in ((qt, 128 * p, p), (kt, C + 128 * p, 4 + p)):
                for th in range(2):
                    pq = msp.tile([128, 512], F32, tag="ms",
                                  name=f"pq_b{b}_p{p}_{bcol}_{th}")
                    for j in range(4):
                        nc.tensor.matmul(
                            pq, wq[j][:, ocol:ocol + 128],
                            xn[j][:, 512 * th:512 * (th + 1)],
                            start=(j == 0), stop=(j == 3))
                    nc.vector.tensor_scalar(
                        out=dst[:, 512 * th:512 * (th + 1)], in0=pq,
                        scalar1=qb_s[:, bcol:bcol + 1], scalar2=None, op0=ALU.add)
            return qt, kt

        a_sb = [apool.tile([128, T], F32R, tag="a", bufs=6,
                           name=f"a_b{b}_{j}") for j in range(4)]
        if mode == 'noattn':
            pairs = []
        else:
            pairs = range(4)
        qknext = make_qk(0) if mode != 'noattn' else None
        for p in pairs:
            qt, kt = qknext
            if p < 3:
                qknext = make_qk(p + 1)
            skip_av = (mode == 'noav')
            # one [65, 512] accumulator per (th, head): rows 0-63 = a,
            # row 64 = softmax denominator (from vt's interleaved ones col)
            avv = [[avp.tile([65, 512], F32, tag="av",
                             name=f"av_b{b}_p{p}_{th}_{hh}")
                    for hh in range(2)] for th in range(2)]
            for st in range(8):
                sce = []
                for th in range(2):
                    ets = []
                    for hh in range(2):
                        sc = scp.tile([128, 512], F32, tag="sc",
                                      name=f"sc_b{b}_p{p}_{st}_{th}_{hh}")
                        nc.tensor.matmul(sc,
                                         kt[64 * hh:64 * hh + 64,
                                            128 * st:128 * (st + 1)],
                                         qt[64 * hh:64 * hh + 64,
                                            512 * th:512 * (th + 1)],
                                         start=True, stop=True)
                        et = ep.tile([128, 512], BF16, tag="e",
                                     name=f"e_b{b}_p{p}_{st}_{th}_{hh}")
                        nc.scalar.activation(out=et, in_=sc, func=AFT.Exp,
                                             scale=SCALE2)
                        ets.append(et)
                    sce.append(ets)
                first, last = (st == 0), (st == 7)
                if skip_av:
                    continue
                for th in range(2):
                    for hh in range(2):
                        nc.tensor.matmul(avv[th][hh],
                                         vt4[:, st, 2 * p + hh, 0:65],
                                         sce[th][hh][:, :],
                                         start=first, stop=last)
            # softmax denominators -> reciprocal -> broadcast -> normalize
            if skip_av:
                continue
            # stage accumulators to SBUF right away so the PSUM banks free
            # up for the next pair; normalize entirely from SBUF afterwards.
            aw = []
            for th in range(2):
                awh = []
                for hh in range(2):
                    a1 = rpool.tile([65, 512], F32, bufs=8, tag="aw",
                                    name=f"aw_b{b}_p{p}_{th}_{hh}")
                    nc.vector.tensor_copy(out=a1, in_=avv[th][hh])
                    awh.append(a1)
                aw.append(awh)
            # gather the 4 Z rows (partition 64 of each), spread across
            # partitions via DMA so the iterative DVE reciprocal is
            # partition-parallel instead of free-dim-serial.
            zr2 = rpool.tile([16, 128], F32, tag="zr2", name=f"zr2_b{b}_p{p}")
            for th in range(2):
                for hh in range(2):
                    k = 2 * th + hh
                    nc.sync.dma_start(
                        out=zr2[4 * k:4 * k + 4, :],
                        in_=aw[th][hh][64:65, :].rearrange("p (a c) -> p a c", a=4))
            r2 = rpool.tile([16, 128], F32, tag="r2", name=f"r2_b{b}_p{p}")
            nc.vector.reciprocal(out=r2, in_=zr2)
            rbd = dramp.tile([4, 512], F32, tag="rbd", name=f"rbd_b{b}_p{p}")
            nc.sync.dma_start(out=rbd, in_=r2)
            for th in range(2):
                rb = rbp.tile([64, 1024], F32, tag="rb", name=f"rb_b{b}_p{p}_{th}")
                nc.sync.dma_start(out=rb[:, 0:512],
                                  in_=rbd[2 * th:2 * th + 1, :].partition_broadcast(64))
                nc.sync.dma_start(out=rb[:, 512:1024],
                                  in_=rbd[2 * th + 1:2 * th + 2, :].partition_broadcast(64))
                nc.vector.tensor_mul(out=a_sb[p][0:64, 512 * th:512 * (th + 1)],
                                     in0=aw[th][0][0:64, :], in1=rb[:, 0:512])
                tb2 = rbp.tile([64, 512], F32R, tag="tb2", name=f"tb2_b{b}_p{p}_{th}")
                nc.vector.tensor_mul(out=tb2, in0=aw[th][1][0:64, :],
                                     in1=rb[:, 512:1024])
                nc.sync.dma_start(out=a_sb[p][64:128, 512 * th:512 * (th + 1)],
                                  in_=tb2)

        # ---- proj + residual -------------------------------------------
        for ot in range(4):
            for th in range(2):
                ph = msp.tile([128, 512], F32, tag="ms", name=f"ph_b{b}_{ot}_{th}")
                nc.tensor.matmul(ph, pb_s[0:1, 128 * ot:128 * (ot + 1)],
                                 ones_row, start=True, stop=False)
                rhs_src = a_sb if mode == 'full' else xn
                for j in range(4):
                    nc.tensor.matmul(ph, wp[j][:, 128 * ot:128 * (ot + 1)],
                                     rhs_src[j][:, 512 * th:512 * (th + 1)],
                                     start=False, stop=(j == 3))
                ob = opool.tile([128, 512], F32, tag="o", name=f"ob_b{b}_{ot}_{th}")
                nc.vector.tensor_add(out=ob, in0=xt[ot][:, 512 * th:512 * (th + 1)],
                                     in1=ph)
                nc.sync.dma_start(
                    out=out_ap[b0, 128 * ot:128 * (ot + 1), 512 * th:512 * (th + 1)],
                    in_=ob)


def build_nc(reps=1, mode='full', loop_iters=0):
    nc = bacc.Bacc("TRN2", target_bir_lowering=False, debug=False)
    x_d = nc.declare_dram_parameter("x", [B_LOC, C, T], F32, isOutput=False)
    wqkvT_d = nc.declare_dram_parameter("wqkvT", [C, 3 * C], F32R, isOutput=False)
    wprojT_d = nc.declare_dram_parameter("wprojT", [C, C], F32R, isOutput=False)
    qb_d = nc.declare_dram_parameter("qb", [128, 8], F32, isOutput=False)
    vb_d = nc.declare_dram_parameter("vb", [128, 512], F32, isOutput=False)
    pb_d = nc.declare_dram_parameter("pb", [1, 512], F32R, isOutput=False)
    nwb_d = nc.declare_dram_parameter("nwb", [128, 8], F32, isOutput=False)
    gmat_d = nc.declare_dram_parameter("gmat", [128, 8], F32R, isOutput=False)
    gbc_d = nc.declare_dram_parameter("gbc", [8, 128], F32R, isOutput=False)
    onesr_d = nc.declare_dram_parameter("onesr", [1, 512], F32R, isOutput=False)
    vones_d = nc.declare_dram_parameter("vones", [128, 64], BF16, isOutput=False)
    out_d = nc.declare_dram_parameter("out", [B_LOC, C, T], F32, isOutput=True)
    dram = {
        "wqkvT": wqkvT_d[:, :], "wprojT": wprojT_d[:, :], "qb": qb_d,
        "vb": vb_d, "pb": pb_d, "nwb": nwb_d, "gmat": gmat_d, "gbc": gbc_d,
        "onesr": onesr_d, "vones": vones_d,
    }
    with tile.TileContext(nc) as tc:
        with (
            tc.tile_pool(name="consts", bufs=1) as consts,
            tc.tile_pool(name="xb", bufs=2) as xb,
            tc.tile_pool(name="xnp", bufs=2) as xnp,
            tc.tile_pool(name="vtp", bufs=2) as vtp,
            tc.tile_pool(name="qkp", bufs=4) as qkp,
            tc.tile_pool(name="ep", bufs=3) as ep,
            tc.tile_pool(name="apool", bufs=1) as apool,
            tc.tile_pool(name="opool", bufs=2) as opool,
            tc.tile_pool(name="rpool", bufs=2) as rpool,
            tc.tile_pool(name="rbp", bufs=3) as rbp,
            tc.tile_pool(name="gnp", bufs=2) as gnp,
            tc.tile_pool(name="scp", bufs=3, space="PSUM") as scp,
            tc.tile_pool(name="avp", bufs=4, space="PSUM") as avp,
            tc.tile_pool(name="msp", bufs=1, space="PSUM") as msp,
            tc.tile_pool(name="dramp", bufs=4, space="DRAM") as dramp,
        ):
            pools = (consts, xb, xnp, vtp, qkp, ep, apool, opool, rpool, rbp,
                     gnp, scp, avp, msp, dramp)
            _emit(nc, tc, pools, x_d[:, :, :], out_d[:, :, :], dram, reps=reps, mode=mode, loop_iters=loop_iters)
    nc.finalize()
    return nc


_CACHE = {}


def _host_inputs(x, norm_w, norm_b, qkv_w, qkv_b, proj_w, proj_b):
    f = lambda a: np.ascontiguousarray(np.asarray(a, dtype=np.float32))
    x = f(x).reshape(B_TOT, C, T)
    qkv_w, qkv_b, proj_w, proj_b = f(qkv_w), f(qkv_b), f(proj_w), f(proj_b)
    norm_w, norm_b = f(norm_w), f(norm_b)
    shared = {
        "wqkvT": np.ascontiguousarray(qkv_w.T),
        "wprojT": np.ascontiguousarray(proj_w.T),
        "qb": np.ascontiguousarray(qkv_b[:1024].reshape(8, 128).T),
        "vb": np.ascontiguousarray(np.broadcast_to(qkv_b[1024:], (128, 512))),
        "pb": np.ascontiguousarray(proj_b.reshape(1, 512)),
        "nwb": np.ascontiguousarray(np.concatenate(
            [norm_w.reshape(4, 128).T, norm_b.reshape(4, 128).T], axis=1)),
    }
    p = np.arange(128)
    gmat = np.zeros((128, 8), np.float32)
    gmat[p, p // 16] = 1.0 / 16.0
    gbc = np.zeros((8, 128), np.float32)
    gbc[p // 16, p] = 1.0
    shared["gmat"] = gmat
    shared["gbc"] = gbc
    shared["onesr"] = np.ones((1, 512), np.float32)
    shared["vones"] = np.ones((128, 64), ml_dtypes.bfloat16)
    in_maps = []
    for i in range(N_CORES):
        m = dict(shared)
        m["x"] = np.ascontiguousarray(x[B_LOC * i:B_LOC * (i + 1)])
        in_maps.append(m)
    return in_maps


def kernel(x, norm_w, norm_b, qkv_w, qkv_b, proj_w, proj_b):
    from concourse.bass_utils import run_bass_kernel_spmd

    if "nc" not in _CACHE:
        _CACHE["nc"] = build_nc()
    in_maps = _host_inputs(x, norm_w, norm_b, qkv_w, qkv_b, proj_w, proj_b)
    res = run_bass_kernel_spmd(_CACHE["nc"], in_maps, core_ids=list(range(N_CORES)))
    out = np.concatenate([r["out"] for r in res.results], axis=0)
    return np.ascontiguousarray(out.reshape(B_TOT, C, 32, 32).astype(np.float32))


if __name__ == "__main__":
    nc = build_nc()
    print("built ok; instructions:",
          sum(len(bb.instructions) for f in nc.m.functions for bb in f.basicblocks)
          if hasattr(nc.m.functions[0], "basicblocks") else "?")
